# revision 1
# baseline (speedup 1.0000x reference)
"""Trainium2 Bass kernel for LocalSelfAttention2d.

Full inputs in, full outputs out. Data-parallel over batch B=16 across 8
NeuronCores (2 images per core). Weights/position table replicated.

Per core, per image:
  - x [256, 4096] raster, 2 chunks of 128 partitions, cast to bf16.
  - q/k projection (w stationary, x moving) -> [o, spatial] layout bf16.
  - v projected transposed (x window-pair stationary, w moving) into
    [position(2 windows), o] layout per window pair.
  - Per window-pair p x 8 heads (16 window-heads per PSUM bank batch):
      scoresT[j,i]: lhsT=k[d,j-win], rhs=q[d,i-win] (K=32, M=64, N=64)
      bias add (DVE, PSUM in-place), exp (ACT) -> attnT bf16 [128, 512]
      oT: lhsT=vT[j,d], rhs=attnT (K=64, M=32, N=64)
      denom: lhsT=ones[64,32], rhs=attnT -> replicated [32, 64]
      norm: oT * recip(denom), scattered into raster o_all (bf16)
  - output projection + per-partition bias -> y fp32, DMA out.
"""
import numpy as np

B, C, H, W = 16, 256, 64, 64
P, HEADS, D = 8, 8, 32
NCORES = 8
B_LOC = B // NCORES  # 2
HW = H * W  # 4096
NW = 8  # windows per row/col

_CACHE = {}


def _rel_bias_np(position):
    coords = np.stack(
        np.meshgrid(np.arange(P), np.arange(P), indexing="ij"), -1
    ).reshape(P * P, 2)
    rel = coords[None, :, :] - coords[:, None, :] + P
    return position[:, rel[..., 0], rel[..., 1]]  # [heads, 64, 64] (h, i, j)


def _wm_scatter(t, wr):
    """Window-major dest AP for raster source of window-row wr.

    t is [128, 4096] window-major (col = win*64 + ph*8 + pw); source cols are
    raster-ordered (ph, ww, pw) within window-row wr. Returns [128, 8, 8, 8]
    AP ordered (ph, ww, pw)."""
    v = t.rearrange("p (wh ww ph pw) -> p wh ph ww pw", wh=NW, ww=NW, ph=P, pw=P)
    return v[:, wr]


def _raster_scatter(t, psq=None):
    """Raster dest AP [128, ww, ph, pw] for window-major-ordered source."""
    return t.rearrange("p (ph ww pw) -> p ww ph pw", ph=P, ww=NW, pw=P)


def _build():
    import concourse.bass as bass  # noqa: F401
    import concourse.tile as tile
    from concourse import bacc, mybir

    f32 = mybir.dt.float32
    bf16 = mybir.dt.bfloat16
    ADD = mybir.AluOpType.add
    MULT = mybir.AluOpType.mult
    EXP = mybir.ActivationFunctionType.Exp

    nc = bacc.Bacc("TRN2", target_bir_lowering=False, debug=False,
                   num_devices=NCORES)

    x_d = nc.dram_tensor("x_sh", [B_LOC, C, HW], f32, kind="ExternalInput").ap()
    wqk_d = nc.dram_tensor("w_qkT", [C, 512], bf16, kind="ExternalInput").ap()
    wv_d = nc.dram_tensor("w_vT", [C, 256], bf16, kind="ExternalInput").ap()
    wo_d = nc.dram_tensor("w_outT", [C, 256], bf16, kind="ExternalInput").ap()
    bias_d = nc.dram_tensor("bias_full", [4, 128, 512], f32,
                            kind="ExternalInput").ap()
    ones_d = nc.dram_tensor("ones_c", [128, 32], bf16, kind="ExternalInput").ap()
    bout_d = nc.dram_tensor("b_out2", [2, 128, 1], f32, kind="ExternalInput").ap()
    y_d = nc.dram_tensor("y_sh", [B_LOC, C, HW], f32, kind="ExternalOutput").ap()

    with tile.TileContext(nc) as tc:
        with (
            tc.tile_pool(name="const", bufs=1) as constp,
            tc.tile_pool(name="xin", bufs=4) as xinp,
            tc.tile_pool(name="xbf", bufs=4) as xbfp,
            tc.tile_pool(name="qkbf", bufs=4) as qkp,
            tc.tile_pool(name="vtbf", bufs=1) as vtp,
            tc.tile_pool(name="att", bufs=4) as attp,
            tc.tile_pool(name="rcp", bufs=4) as rcpp,
            tc.tile_pool(name="oall", bufs=2) as oallp,
            tc.tile_pool(name="ysb", bufs=4) as ysbp,
            tc.tile_pool(name="psum", bufs=8, space="PSUM") as psp,
        ):
            # ---- constants (loaded once) ----
            wqk_sb = []
            wv_sb = []
            wo_sb = []
            bout_sb = []
            for kc in range(2):
                t = constp.tile([128, 512], bf16, tag=f"wqk{kc}", name=f"wqk{kc}")
                nc.sync.dma_start(t[:], wqk_d[kc * 128 : (kc + 1) * 128, :])
                wqk_sb.append(t)
                t = constp.tile([128, 256], bf16, tag=f"wv{kc}", name=f"wv{kc}")
                nc.sync.dma_start(t[:], wv_d[kc * 128 : (kc + 1) * 128, :])
                wv_sb.append(t)
                t = constp.tile([128, 256], bf16, tag=f"wo{kc}", name=f"wo{kc}")
                nc.sync.dma_start(t[:], wo_d[kc * 128 : (kc + 1) * 128, :])
                wo_sb.append(t)
                t = constp.tile([128, 1], f32, tag=f"bo{kc}", name=f"bo{kc}")
                nc.sync.dma_start(t[:], bout_d[kc])
                bout_sb.append(t)
            bias_sb = []
            for r in range(4):
                t = constp.tile([128, 512], f32, tag=f"bias{r}", name=f"bias{r}")
                nc.sync.dma_start(t[:], bias_d[r])
                bias_sb.append(t)
            ones_sb = constp.tile([128, 32], bf16, tag="ones", name="ones")
            nc.sync.dma_start(ones_sb[:], ones_d[:])

            for b in range(B_LOC):
                # ---- phase A: load x, cast to bf16 + window-major scatter ----
                x_bf = []
                for cc in range(2):
                    xb = xbfp.tile([128, HW], bf16, tag="xbf", name="xbf")
                    for t4 in range(4):
                        xi = xinp.tile([128, 1024], f32, tag="xin", name="xin")
                        nc.sync.dma_start(
                            xi[:],
                            x_d[b, cc * 128 : (cc + 1) * 128,
                                t4 * 1024 : (t4 + 1) * 1024],
                        )
                        for u in range(2):
                            src = xi[:, u * 512 : (u + 1) * 512].rearrange(
                                "p (ph ww pw) -> p ph ww pw", ph=P, ww=NW, pw=P
                            )
                            nc.gpsimd.tensor_copy(
                                _wm_scatter(xb, 2 * t4 + u), src
                            )
                    x_bf.append(xb)

                # ---- phase B: q/k projection -> qk_bf[4][128, 4096] ----
                qk_bf = [qkp.tile([128, HW], bf16, tag="qkbf", name="qkbf") for _ in range(4)]
                for mc in range(4):
                    for nt in range(8):
                        ps = psp.tile([128, 512], f32, tag="bank", name="bank")
                        for kc in range(2):
                            nc.tensor.matmul(
                                ps[:],
                                lhsT=wqk_sb[kc][:, mc * 128 : (mc + 1) * 128],
                                rhs=x_bf[kc][:, nt * 512 : (nt + 1) * 512],
                                start=(kc == 0),
                                stop=(kc == 1),
                            )
                        nc.vector.tensor_copy(
                            qk_bf[mc][:, nt * 512 : (nt + 1) * 512], ps[:]
                        )

                # ---- phase C: vT projection -> vT_bf [128, 8192] ----
                # window pair p: windows (2p, 2p+1); vT rows = [j win0 | j win1]
                vt_bf = vtp.tile([128, 32 * 256], bf16, tag="vt", name="vt")
                for p in range(32):
                    ps = psp.tile([128, 256], f32, tag="bank", name="bank")
                    for kc in range(2):
                        nc.tensor.matmul(
                            ps[:],
                            lhsT=x_bf[kc][:, p * 128 : (p + 1) * 128],
                            rhs=wv_sb[kc][:],
                            start=(kc == 0),
                            stop=(kc == 1),
                        )
                    nc.vector.tensor_copy(
                        vt_bf[:, p * 256 : (p + 1) * 256], ps[:]
                    )

                # ---- phase D: attention per window pair ----
                o_all = [oallp.tile([128, HW], bf16, tag="oall", name="oall") for _ in range(2)]
                # unit = (pair-block pb of 4 pairs) x (row strip r) x (parity c)
                # every PSUM bank sees a single constant tile_position.
                for pb in range(8):
                    for r in range(4):
                        for c in range(2):
                            ps_s = psp.tile([128, 512], f32, tag="bank",
                                            name="bank")
                            for u in range(4):
                                win = 2 * (pb * 4 + u) + c
                                for hg in range(2):
                                    kch = qk_bf[2 + hg]
                                    qch = qk_bf[hg]
                                    col = u * 128 + hg * 64
                                    nc.tensor.matmul(
                                        ps_s[c * 64 : (c + 1) * 64,
                                             col : col + 64],
                                        lhsT=kch[r * 32 : (r + 1) * 32,
                                                 win * 64 : (win + 1) * 64],
                                        rhs=qch[r * 32 : (r + 1) * 32,
                                                win * 64 : (win + 1) * 64],
                                        start=True, stop=True,
                                        tile_position=(r * 32, c * 64),
                                    )
                            # bias add on the used half, then exp -> bf16
                            nc.vector.tensor_tensor(
                                ps_s[c * 64 : (c + 1) * 64, :],
                                ps_s[c * 64 : (c + 1) * 64, :],
                                bias_sb[r][c * 64 : (c + 1) * 64, :], ADD)
                            at = attp.tile([128, 512], bf16, tag="att",
                                           name="att")
                            nc.scalar.activation(
                                at[c * 64 : (c + 1) * 64, :],
                                ps_s[c * 64 : (c + 1) * 64, :], EXP)
                            ps_o = psp.tile([128, 512], f32, tag="bank",
                                            name="bank")
                            ps_d = psp.tile([128, 512], f32, tag="bank",
                                            name="bank")
                            for u in range(4):
                                p = pb * 4 + u
                                for hg in range(2):
                                    h = r + 4 * hg
                                    col = u * 128 + hg * 64
                                    atap = at[c * 64 : (c + 1) * 64,
                                              col : col + 64]
                                    vap = vt_bf[
                                        c * 64 : (c + 1) * 64,
                                        p * 256 + h * 32 : p * 256 + (h + 1) * 32]
                                    nc.tensor.matmul(
                                        ps_o[r * 32 : (r + 1) * 32,
                                             col : col + 64],
                                        lhsT=vap, rhs=atap,
                                        start=True, stop=True,
                                        tile_position=(c * 64, r * 32),
                                    )
                                    nc.tensor.matmul(
                                        ps_d[r * 32 : (r + 1) * 32,
                                             col : col + 64],
                                        lhsT=ones_sb[c * 64 : (c + 1) * 64, :],
                                        rhs=atap,
                                        start=True, stop=True,
                                        tile_position=(c * 64, r * 32),
                                    )
                            rc = rcpp.tile([32, 512], f32, tag="rcp",
                                           name="rcp")
                            nc.vector.reciprocal(
                                rc[:], ps_d[r * 32 : (r + 1) * 32, :])
                            # normalize into window-major o_all
                            for hg in range(2):
                                src = ps_o[r * 32 : (r + 1) * 32, :].rearrange(
                                    "p (u hg i) -> p hg u i", u=4, hg=2, i=64
                                )[:, hg]
                                rcv = rc[:].rearrange(
                                    "p (u hg i) -> p hg u i", u=4, hg=2, i=64
                                )[:, hg]
                                oa = o_all[hg].rearrange(
                                    "p (pb u c i) -> p pb c u i",
                                    pb=8, u=4, c=2, i=64)
                                dst = oa[r * 32 : (r + 1) * 32, pb, c]
                                nc.vector.tensor_tensor(dst, src, rcv, MULT)

                # ---- phase E: output projection ----
                for mc in range(2):
                    for nt in range(8):
                        ps = psp.tile([128, 512], f32, tag="bank", name="bank")
                        for kc in range(2):
                            nc.tensor.matmul(
                                ps[:],
                                lhsT=wo_sb[kc][:, mc * 128 : (mc + 1) * 128],
                                rhs=o_all[kc][:, nt * 512 : (nt + 1) * 512],
                                start=(kc == 0),
                                stop=(kc == 1),
                            )
                        yt = ysbp.tile([128, 512], f32, tag="ysb", name="ysb")
                        psv = ps.rearrange(
                            "p (ww ph pw) -> p ww ph pw", ww=NW, ph=P, pw=P
                        )
                        nc.vector.tensor_scalar_add(
                            _raster_scatter(yt), psv, bout_sb[mc][:]
                        )
                        nc.sync.dma_start(
                            y_d[b, mc * 128 : (mc + 1) * 128,
                                nt * 512 : (nt + 1) * 512],
                            yt[:],
                        )

    nc.compile()
    return nc


def _prep_consts(w_proj, position, w_out, b_out):
    scale = 1.0 / np.sqrt(np.float32(D))
    w_qkT = np.ascontiguousarray(w_proj[:512].T).astype(np.float32)
    w_qkT[:, :256] *= scale
    import ml_dtypes
    bf16 = ml_dtypes.bfloat16
    w_qkT = w_qkT.astype(bf16)
    w_vT = np.ascontiguousarray(w_proj[512:].T).astype(bf16)
    w_outT = np.ascontiguousarray(w_out.T).astype(bf16)
    bias = _rel_bias_np(position.astype(np.float32))  # [h, i, j]
    # bias_full[r][c*64+j, u*128 + hg*64 + i] = biasT[r+4*hg][j, i]
    bf = np.empty((4, 128, 512), np.float32)
    for r in range(4):
        for hg in range(2):
            bT = bias[r + 4 * hg].T  # [j, i]
            for u in range(4):
                col = u * 128 + hg * 64
                bf[r, :64, col : col + 64] = bT
                bf[r, 64:, col : col + 64] = bT
    ones_c = np.ones((128, 32), bf16)
    b_out2 = np.ascontiguousarray(
        b_out.astype(np.float32).reshape(2, 128, 1)
    )
    return {
        "w_qkT": w_qkT,
        "w_vT": w_vT,
        "w_outT": w_outT,
        "bias_full": bf,
        "ones_c": ones_c,
        "b_out2": b_out2,
    }


def kernel(x, w_proj, position, w_out, b_out):
    from concourse.bass_utils import run_bass_kernel_spmd

    if "nc" not in _CACHE:
        _CACHE["nc"] = _build()
    nc = _CACHE["nc"]

    consts = _prep_consts(w_proj, position, w_out, b_out)
    x = np.asarray(x, np.float32).reshape(B, C, HW)
    in_maps = []
    for i in range(NCORES):
        m = dict(consts)
        m["x_sh"] = np.ascontiguousarray(x[i * B_LOC : (i + 1) * B_LOC])
        in_maps.append(m)

    res = run_bass_kernel_spmd(nc, in_maps, core_ids=list(range(NCORES)))
    out = np.concatenate([res.results[i]["y_sh"] for i in range(NCORES)], axis=0)
    return out.reshape(B, C, H, W)

